# revision 13
# baseline (speedup 1.0000x reference)
"""Trainium2 Bass kernel for RoPE + scaled-dot-product attention (returns out AND attn).

Reference semantics (B=2, H=16, T=2048, D=64, fp32):
    qr, kr = rope(q), rope(k)
    scores = qr @ kr^T / sqrt(D)          # [B,H,T,T]
    attn   = softmax(scores, axis=-1)     # output 1
    out    = attn @ v                     # output 2

Sharding: B*H = 32 (b,h) pairs, 4 per NeuronCore across 8 cores; no
cross-core communication.

Per-head on-device plan (T=2048, D=64), v2:
  phase 0: load q,k,v; RoPE on DVE in natural [t,d] layout; TensorE
           transposes with a stride-0 broadcast AP produce Qr^T/Kr^T
           DUPLICATED on both partition halves ([128, T], rows 0-63 ==
           rows 64-127) so S and S^T matmuls run on disjoint PE row
           groups concurrently.
  phase 1 (rows 0-63):  S = Qr^T-blk.T @ Kr^T (float32r), exp on ScalarE
           with fused accum_out row-sums, attn = E * (1/rowsum) split
           across DVE and GpSimd, 1MB attn DMA per t-block.
  phase 2 (rows 64-127): S^T s-blocks, exp#2 -> E^T, out^T accumulated as
           V-blk.T @ E^T with column-group packing (j-parity -> PE col
           groups 0/1), then transpose out^T back and scale by 1/rowsum.
  Phases 1 and 2 are independent given phase 0; emission interleaves them
  so TensorE keeps both row-group streams and ScalarE both exp streams.
"""

import os
import sys

import numpy as np

for _p in ("/opt/trn_rl_repo", "/root/.axon_site/_ro/trn_rl_repo"):
    if os.path.isdir(_p) and _p not in sys.path:
        sys.path.append(_p)

import concourse.bacc as bacc
import concourse.bass as bass
import concourse.mybir as mybir
import concourse.tile as tile
from concourse.masks import make_identity

F32 = mybir.dt.float32
F32R = mybir.dt.float32r
F16 = mybir.dt.float16
EXP = mybir.ActivationFunctionType.Exp

N_CORES = 8
B, H, T, D = 2, 16, 2048, 64
HEADS_PER_CORE = (B * H) // N_CORES  # 4
SCALE = 1.0 / np.sqrt(np.float32(D))  # 1/8


def _bcast2(ap):
    """[P, F] AP read twice along free dim -> [P, 2, F] with stride-0 mid."""
    return bass.AP(tensor=ap.tensor, offset=ap.offset,
                   ap=[ap.ap[0], [0, 2], ap.ap[1]])


def build_nc(heads=HEADS_PER_CORE, t=T, d=D):
    """Emit the per-core Bass program. All shapes static."""
    P = 128
    TB = t // P           # 128-row blocks (16)
    MM_N = min(512, t)    # matmul moving free dim
    BLK = min(1024, t)    # free-dim block for exp / S tiles
    G = t // BLK          # s-halves per t-block
    NCH = BLK // MM_N     # matmul chunks per BLK
    dh = d // 2

    nc = bacc.Bacc("TRN2", target_bir_lowering=False, debug=False)
    q_d = nc.dram_tensor("q", [heads, t, d], F32, kind="ExternalInput")
    k_d = nc.dram_tensor("k", [heads, t, d], F32, kind="ExternalInput")
    v_d = nc.dram_tensor("v", [heads, t, d], F32, kind="ExternalInput")
    cos_d = nc.dram_tensor("cos_half", [t, dh], F32, kind="ExternalInput")
    sin_d = nc.dram_tensor("sin_half", [t, dh], F32, kind="ExternalInput")
    attn_d = nc.dram_tensor("attn", [heads, t, t], F32, kind="ExternalOutput")
    out_d = nc.dram_tensor("out", [heads, t, d], F32, kind="ExternalOutput")

    with tile.TileContext(nc) as tc:
        with (
            tc.tile_pool(name="singles", bufs=1) as singles,
            tc.tile_pool(name="nat", bufs=2) as p_nat,
            tc.tile_pool(name="tmp", bufs=2) as p_tmp,
            tc.tile_pool(name="tmat", bufs=2) as p_tmat,
            tc.tile_pool(name="e", bufs=4) as p_e,
            tc.tile_pool(name="et", bufs=3) as p_et,
            tc.tile_pool(name="attn", bufs=4) as p_attn,
            tc.tile_pool(name="small", bufs=4) as p_small,
            tc.tile_pool(name="head", bufs=2) as p_head,
            tc.tile_pool(name="outb", bufs=2) as p_out,
            tc.tile_pool(name="mm", bufs=2, space="PSUM") as p_mm,
            tc.tile_pool(name="av", bufs=2, space="PSUM") as p_av,
            tc.tile_pool(name="tr", bufs=2, space="PSUM") as p_tr,
        ):
            ident = singles.tile([P, P], F32)
            make_identity(nc, ident)

            cos_sb = singles.tile([P, TB, dh], F32)
            sin_sb = singles.tile([P, TB, dh], F32)
            nc.gpsimd.dma_start(
                out=cos_sb, in_=cos_d.rearrange("(i p) d -> p i d", p=P))
            nc.gpsimd.dma_start(
                out=sin_sb, in_=sin_d.rearrange("(i p) d -> p i d", p=P))

            for h in range(heads):
                # ---------------- phase 0: load + rope + transpose ----------
                q_nat = p_nat.tile([P, TB, d], F32, tag="q_nat")
                k_nat = p_nat.tile([P, TB, d], F32, tag="k_nat")
                v_sb = p_nat.tile([P, TB, d], F16, tag="v_sb")
                nc.gpsimd.dma_start(
                    out=q_nat, in_=q_d[h].rearrange("(i p) d -> p i d", p=P))
                nc.gpsimd.dma_start(
                    out=k_nat, in_=k_d[h].rearrange("(i p) d -> p i d", p=P))
                nc.gpsimd.dma_start(
                    out=v_sb, in_=v_d[h].rearrange("(i p) d -> p i d", p=P))

                qr_nat = p_nat.tile([P, TB, d], F32, tag="qr_nat")
                kr_nat = p_nat.tile([P, TB, d], F32, tag="kr_nat")
                for src, dst, tg in ((q_nat, qr_nat, "tq"), (k_nat, kr_nat, "tk")):
                    x1 = src[:, :, 0:dh]
                    x2 = src[:, :, dh:d]
                    tmp = p_tmp.tile([P, TB, dh], F32, tag=tg)
                    nc.vector.tensor_mul(dst[:, :, 0:dh], x1, cos_sb)
                    nc.vector.tensor_mul(tmp, x2, sin_sb)
                    nc.vector.tensor_sub(dst[:, :, 0:dh], dst[:, :, 0:dh], tmp)
                    nc.vector.tensor_mul(dst[:, :, dh:d], x1, sin_sb)
                    nc.vector.tensor_mul(tmp, x2, cos_sb)
                    nc.vector.tensor_add(dst[:, :, dh:d], dst[:, :, dh:d], tmp)

                # Qr^T / Kr^T duplicated on both partition halves: PE
                # transpose into rows 0-63, then one SB->SB DMA up-copy.
                qrT = p_tmat.tile([P, t], F32R, tag="qrT")
                krT = p_tmat.tile([P, t], F32R, tag="krT")
                for mat, matT in ((qr_nat, qrT), (kr_nat, krT)):
                    for i in range(TB):
                        trp = p_tr.tile([P, P], F32, tag="tr")
                        nc.tensor.transpose(trp[0:d, :], mat[:, i, :], ident)
                        nc.vector.tensor_copy(
                            matT[0:d, i * P:(i + 1) * P], trp[0:d, :])
                    nc.sync.dma_start(out=matT[d:2 * d, :], in_=matT[0:d, :])

                recip_all = p_head.tile([P, TB], F32, tag="recip")

                # Interleave phase-1 t-blocks with phase-2 (g2, j) steps.
                def p1_block(i):
                    racc = p_small.tile([P, max(G, 2)], F32, tag="racc")
                    e_blocks = []
                    for g in range(G):
                        s_ps = p_mm.tile([P, BLK], F32, tag="mm")
                        for c in range(NCH):
                            nc.tensor.matmul(
                                s_ps[:, c * MM_N:(c + 1) * MM_N],
                                qrT[0:d, i * P:(i + 1) * P],
                                krT[0:d, g * BLK + c * MM_N: g * BLK + (c + 1) * MM_N],
                                start=True, stop=True)
                        e_sb = p_e.tile([P, BLK], F32, tag="e")
                        nc.scalar.activation(
                            e_sb, s_ps, EXP, scale=float(SCALE),
                            accum_out=racc[:, g:g + 1])
                        e_blocks.append(e_sb)
                    r_i = p_small.tile([P, 1], F32, tag="r")
                    if G == 1:
                        nc.vector.reciprocal(recip_all[:, i:i + 1], racc[:, 0:1])
                    else:
                        nc.vector.tensor_add(r_i, racc[:, 0:1], racc[:, 1:2])
                        for g in range(2, G):
                            nc.vector.tensor_add(r_i, r_i, racc[:, g:g + 1])
                        nc.vector.reciprocal(recip_all[:, i:i + 1], r_i)
                    attn_sb = p_attn.tile([P, t], F32, tag="attn")
                    for g in range(G):
                        eng = nc.vector if (g % 2 == 0) else nc.gpsimd
                        eng.tensor_scalar_mul(
                            attn_sb[:, g * BLK:(g + 1) * BLK], e_blocks[g],
                            recip_all[:, i:i + 1])
                    nc.sync.dma_start(
                        out=attn_d[h, i * P:(i + 1) * P, :], in_=attn_sb)

                av_state = {}

                def p2_step(w):
                    g2, j = divmod(w, TB)
                    if j == 0:
                        av_state["av"] = [
                            p_av.tile([P, MM_N], F32, tag="av",
                                      name=f"av_ps{c}")
                            for c in range(NCH)]
                    st_ps = p_mm.tile([P, BLK], F32, tag="mm")
                    for c in range(NCH):
                        nc.tensor.matmul(
                            st_ps[:, c * MM_N:(c + 1) * MM_N],
                            krT[64:64 + d, j * P:(j + 1) * P],
                            qrT[64:64 + d,
                                g2 * BLK + c * MM_N: g2 * BLK + (c + 1) * MM_N],
                            start=True, stop=True)
                    et_sb = p_et.tile([P, BLK], F16, tag="et")
                    nc.scalar.activation(et_sb, st_ps, EXP, scale=float(SCALE))
                    half = j % 2
                    for c in range(NCH):
                        nc.tensor.matmul(
                            av_state["av"][c][half * d:(half + 1) * d, :],
                            v_sb[:, j, :],
                            et_sb[:, c * MM_N:(c + 1) * MM_N],
                            start=(j < 2), stop=(j >= TB - 2),
                            tile_position=(0, half * 64))
                    if j == TB - 1:
                        outT_sb = p_out.tile([d, BLK], F32, tag="outT")
                        for c in range(NCH):
                            sl = slice(c * MM_N, (c + 1) * MM_N)
                            nc.vector.tensor_copy(
                                outT_sb[:, sl], av_state["av"][c][0:d, :])
                            if TB > 1:
                                nc.vector.tensor_add(
                                    outT_sb[:, sl], outT_sb[:, sl],
                                    av_state["av"][c][d:2 * d, :])
                        nblk = BLK // P
                        out_sb = p_out.tile([P, nblk, d], F32, tag="out_sb")
                        for tt in range(nblk):
                            trp2 = p_tr.tile([P, P], F32, tag="tr")
                            nc.tensor.transpose(
                                trp2[:, 0:d], outT_sb[:, tt * P:(tt + 1) * P],
                                ident[0:d, 0:d])
                            iglob = g2 * nblk + tt
                            nc.vector.tensor_scalar_mul(
                                out_sb[:, tt, :], trp2[:, 0:d],
                                recip_all[:, iglob:iglob + 1])
                        nc.sync.dma_start(
                            out=out_d[h, g2 * BLK:(g2 + 1) * BLK, :].rearrange(
                                "(tt p) d -> p tt d", p=P),
                            in_=out_sb)

                n_p2 = G * TB
                for z in range(TB):
                    p1_block(z)
                    w0 = z * n_p2 // TB
                    w1 = (z + 1) * n_p2 // TB
                    for w in range(w0, w1):
                        p2_step(w)
    nc.compile()
    return nc


def rope_tables(t=T, d=D):
    inv_freq = (1.0 / (10000.0 ** (np.arange(0, d, 2, dtype=np.float32) / np.float32(d)))).astype(np.float32)
    pos = np.arange(t, dtype=np.float32)
    freqs = pos[:, None] * inv_freq[None, :]  # [t, d//2] fp32
    return np.cos(freqs).astype(np.float32), np.sin(freqs).astype(np.float32)


_CACHED = {}


def kernel(q, k, v):
    """Full-input entry point: q,k,v [2,16,2048,64] fp32 -> (out, attn)."""
    from concourse.bass_utils import run_bass_kernel_spmd

    q = np.ascontiguousarray(q, dtype=np.float32)
    k = np.ascontiguousarray(k, dtype=np.float32)
    v = np.ascontiguousarray(v, dtype=np.float32)

    if "nc" not in _CACHED:
        _CACHED["nc"] = build_nc()
        _CACHED["tables"] = rope_tables()
    nc = _CACHED["nc"]
    cos_half, sin_half = _CACHED["tables"]

    qf = q.reshape(B * H, T, D)
    kf = k.reshape(B * H, T, D)
    vf = v.reshape(B * H, T, D)
    in_maps = []
    for c in range(N_CORES):
        sl = slice(c * HEADS_PER_CORE, (c + 1) * HEADS_PER_CORE)
        in_maps.append({
            "q": qf[sl], "k": kf[sl], "v": vf[sl],
            "cos_half": cos_half, "sin_half": sin_half,
        })
    res = run_bass_kernel_spmd(nc, in_maps, list(range(N_CORES)))
    outs = np.concatenate([r["out"] for r in res.results], axis=0)
    attns = np.concatenate([r["attn"] for r in res.results], axis=0)
    return (outs.reshape(B, H, T, D), attns.reshape(B, H, T, T))


if __name__ == "__main__":
    rng = np.random.default_rng(0)
    q = rng.standard_normal((B, H, T, D), dtype=np.float32)
    k = rng.standard_normal((B, H, T, D), dtype=np.float32)
    v = rng.standard_normal((B, H, T, D), dtype=np.float32)
    out, attn = kernel(q, k, v)
    print(out.shape, attn.shape, float(attn[0, 0, 0].sum()))


# revision 14
# speedup vs baseline: 1.5664x; 1.5664x over previous
"""Trainium2 Bass kernel for RoPE + scaled-dot-product attention (returns out AND attn).

Reference semantics (B=2, H=16, T=2048, D=64, fp32):
    qr, kr = rope(q), rope(k)
    scores = qr @ kr^T / sqrt(D)          # [B,H,T,T]
    attn   = softmax(scores, axis=-1)     # output 1
    out    = attn @ v                     # output 2

Sharding: B*H = 32 (b,h) pairs, 4 per NeuronCore across 8 cores; no
cross-core communication.

Per-head on-device plan (T=2048, D=64), v2:
  phase 0: load q,k,v; RoPE on DVE in natural [t,d] layout; TensorE
           transposes with a stride-0 broadcast AP produce Qr^T/Kr^T
           DUPLICATED on both partition halves ([128, T], rows 0-63 ==
           rows 64-127) so S and S^T matmuls run on disjoint PE row
           groups concurrently.
  phase 1 (rows 0-63):  S = Qr^T-blk.T @ Kr^T (float32r), exp on ScalarE
           with fused accum_out row-sums, attn = E * (1/rowsum) split
           across DVE and GpSimd, 1MB attn DMA per t-block.
  phase 2 (rows 64-127): S^T s-blocks, exp#2 -> E^T, out^T accumulated as
           V-blk.T @ E^T with column-group packing (j-parity -> PE col
           groups 0/1), then transpose out^T back and scale by 1/rowsum.
  Phases 1 and 2 are independent given phase 0; emission interleaves them
  so TensorE keeps both row-group streams and ScalarE both exp streams.
"""

import os
import sys

import numpy as np

for _p in ("/opt/trn_rl_repo", "/root/.axon_site/_ro/trn_rl_repo"):
    if os.path.isdir(_p) and _p not in sys.path:
        sys.path.append(_p)

import concourse.bacc as bacc
import concourse.bass as bass
import concourse.mybir as mybir
import concourse.tile as tile
from concourse.masks import make_identity

F32 = mybir.dt.float32
F32R = mybir.dt.float32r
F16 = mybir.dt.float16
EXP = mybir.ActivationFunctionType.Exp

N_CORES = 8
B, H, T, D = 2, 16, 2048, 64
HEADS_PER_CORE = (B * H) // N_CORES  # 4
SCALE = 1.0 / np.sqrt(np.float32(D))  # 1/8


def _bcast2(ap):
    """[P, F] AP read twice along free dim -> [P, 2, F] with stride-0 mid."""
    return bass.AP(tensor=ap.tensor, offset=ap.offset,
                   ap=[ap.ap[0], [0, 2], ap.ap[1]])


def build_nc(heads=HEADS_PER_CORE, t=T, d=D):
    """Emit the per-core Bass program. All shapes static."""
    P = 128
    TB = t // P           # 128-row blocks (16)
    MM_N = min(512, t)    # matmul moving free dim
    BLK = min(1024, t)    # free-dim block for exp / S tiles
    G = t // BLK          # s-halves per t-block
    NCH = BLK // MM_N     # matmul chunks per BLK
    dh = d // 2

    nc = bacc.Bacc("TRN2", target_bir_lowering=False, debug=False)
    q_d = nc.dram_tensor("q", [heads, t, d], F32, kind="ExternalInput")
    k_d = nc.dram_tensor("k", [heads, t, d], F32, kind="ExternalInput")
    v_d = nc.dram_tensor("v", [heads, t, d], F32, kind="ExternalInput")
    cos_d = nc.dram_tensor("cos_half", [t, dh], F32, kind="ExternalInput")
    sin_d = nc.dram_tensor("sin_half", [t, dh], F32, kind="ExternalInput")
    attn_d = nc.dram_tensor("attn", [heads, t, t], F32, kind="ExternalOutput")
    out_d = nc.dram_tensor("out", [heads, t, d], F32, kind="ExternalOutput")

    with tile.TileContext(nc) as tc:
        with (
            tc.tile_pool(name="singles", bufs=1) as singles,
            tc.tile_pool(name="nat", bufs=2) as p_nat,
            tc.tile_pool(name="tmp", bufs=2) as p_tmp,
            tc.tile_pool(name="tmat", bufs=2) as p_tmat,
            tc.tile_pool(name="e", bufs=4) as p_e,
            tc.tile_pool(name="et", bufs=3) as p_et,
            tc.tile_pool(name="attn", bufs=4) as p_attn,
            tc.tile_pool(name="small", bufs=4) as p_small,
            tc.tile_pool(name="head", bufs=2) as p_head,
            tc.tile_pool(name="outb", bufs=2) as p_out,
            tc.tile_pool(name="mm", bufs=2, space="PSUM") as p_mm,
            tc.tile_pool(name="av", bufs=2, space="PSUM") as p_av,
            tc.tile_pool(name="tr", bufs=2, space="PSUM") as p_tr,
        ):
            ident = singles.tile([P, P], F32)
            make_identity(nc, ident)

            cos_sb = singles.tile([P, TB, dh], F32)
            sin_sb = singles.tile([P, TB, dh], F32)
            nc.gpsimd.dma_start(
                out=cos_sb, in_=cos_d.rearrange("(i p) d -> p i d", p=P))
            nc.gpsimd.dma_start(
                out=sin_sb, in_=sin_d.rearrange("(i p) d -> p i d", p=P))

            for h in range(heads):
                # ---------------- phase 0: load + rope + transpose ----------
                q_nat = p_nat.tile([P, TB, d], F32, tag="q_nat")
                k_nat = p_nat.tile([P, TB, d], F32, tag="k_nat")
                v_sb = p_nat.tile([P, TB, d], F16, tag="v_sb")
                nc.gpsimd.dma_start(
                    out=q_nat, in_=q_d[h].rearrange("(i p) d -> p i d", p=P))
                nc.gpsimd.dma_start(
                    out=k_nat, in_=k_d[h].rearrange("(i p) d -> p i d", p=P))
                nc.gpsimd.dma_start(
                    out=v_sb, in_=v_d[h].rearrange("(i p) d -> p i d", p=P))

                qr_nat = p_nat.tile([P, TB, d], F32, tag="qr_nat")
                kr_nat = p_nat.tile([P, TB, d], F32, tag="kr_nat")
                for src, dst, tg in ((q_nat, qr_nat, "tq"), (k_nat, kr_nat, "tk")):
                    x1 = src[:, :, 0:dh]
                    x2 = src[:, :, dh:d]
                    tmp = p_tmp.tile([P, TB, dh], F32, tag=tg)
                    nc.vector.tensor_mul(dst[:, :, 0:dh], x1, cos_sb)
                    nc.vector.tensor_mul(tmp, x2, sin_sb)
                    nc.vector.tensor_sub(dst[:, :, 0:dh], dst[:, :, 0:dh], tmp)
                    nc.vector.tensor_mul(dst[:, :, dh:d], x1, sin_sb)
                    nc.vector.tensor_mul(tmp, x2, cos_sb)
                    nc.vector.tensor_add(dst[:, :, dh:d], dst[:, :, dh:d], tmp)

                # Qr^T / Kr^T duplicated on both partition halves: PE
                # transpose into rows 0-63, then one SB->SB DMA up-copy.
                qrT = p_tmat.tile([P, t], F32R, tag="qrT")
                krT = p_tmat.tile([P, t], F32R, tag="krT")
                for mat, matT in ((qr_nat, qrT), (kr_nat, krT)):
                    for i in range(TB):
                        trp = p_tr.tile([P, P], F32, tag="tr")
                        nc.tensor.transpose(trp[0:d, :], mat[:, i, :], ident)
                        nc.vector.tensor_copy(
                            matT[0:d, i * P:(i + 1) * P], trp[0:d, :])
                    nc.sync.dma_start(out=matT[d:2 * d, :], in_=matT[0:d, :])

                recip_all = p_head.tile([P, TB], F32, tag="recip")

                # Interleave phase-1 t-blocks with phase-2 (g2, j) steps.
                def p1_block(i):
                    racc = p_small.tile([P, max(G, 2)], F32, tag="racc")
                    e_blocks = []
                    for g in range(G):
                        s_ps = p_mm.tile([P, BLK], F32, tag="mm")
                        for c in range(NCH):
                            nc.tensor.matmul(
                                s_ps[:, c * MM_N:(c + 1) * MM_N],
                                qrT[0:d, i * P:(i + 1) * P],
                                krT[0:d, g * BLK + c * MM_N: g * BLK + (c + 1) * MM_N],
                                start=True, stop=True)
                        e_sb = p_e.tile([P, BLK], F32, tag="e")
                        nc.scalar.activation(
                            e_sb, s_ps, EXP, scale=float(SCALE),
                            accum_out=racc[:, g:g + 1])
                        e_blocks.append(e_sb)
                    r_i = p_small.tile([P, 1], F32, tag="r")
                    if G == 1:
                        nc.vector.reciprocal(recip_all[:, i:i + 1], racc[:, 0:1])
                    else:
                        nc.vector.tensor_add(r_i, racc[:, 0:1], racc[:, 1:2])
                        for g in range(2, G):
                            nc.vector.tensor_add(r_i, r_i, racc[:, g:g + 1])
                        nc.vector.reciprocal(recip_all[:, i:i + 1], r_i)
                    attn_sb = p_attn.tile([P, t], F32, tag="attn")
                    for g in range(G):
                        nc.vector.tensor_scalar_mul(
                            attn_sb[:, g * BLK:(g + 1) * BLK], e_blocks[g],
                            recip_all[:, i:i + 1])
                    nc.sync.dma_start(
                        out=attn_d[h, i * P:(i + 1) * P, :], in_=attn_sb)

                av_state = {}

                def p2_step(w):
                    g2, j = divmod(w, TB)
                    if j == 0:
                        av_state["av"] = [
                            p_av.tile([P, MM_N], F32, tag="av",
                                      name=f"av_ps{c}")
                            for c in range(NCH)]
                    st_ps = p_mm.tile([P, BLK], F32, tag="mm")
                    for c in range(NCH):
                        nc.tensor.matmul(
                            st_ps[:, c * MM_N:(c + 1) * MM_N],
                            krT[64:64 + d, j * P:(j + 1) * P],
                            qrT[64:64 + d,
                                g2 * BLK + c * MM_N: g2 * BLK + (c + 1) * MM_N],
                            start=True, stop=True)
                    et_sb = p_et.tile([P, BLK], F16, tag="et")
                    nc.scalar.activation(et_sb, st_ps, EXP, scale=float(SCALE))
                    half = j % 2
                    for c in range(NCH):
                        nc.tensor.matmul(
                            av_state["av"][c][half * d:(half + 1) * d, :],
                            v_sb[:, j, :],
                            et_sb[:, c * MM_N:(c + 1) * MM_N],
                            start=(j < 2), stop=(j >= TB - 2),
                            tile_position=(0, half * 64))
                    if j == TB - 1:
                        outT_sb = p_out.tile([d, BLK], F32, tag="outT")
                        for c in range(NCH):
                            sl = slice(c * MM_N, (c + 1) * MM_N)
                            nc.vector.tensor_copy(
                                outT_sb[:, sl], av_state["av"][c][0:d, :])
                            if TB > 1:
                                nc.vector.tensor_add(
                                    outT_sb[:, sl], outT_sb[:, sl],
                                    av_state["av"][c][d:2 * d, :])
                        nblk = BLK // P
                        out_sb = p_out.tile([P, nblk, d], F32, tag="out_sb")
                        for tt in range(nblk):
                            trp2 = p_tr.tile([P, P], F32, tag="tr")
                            nc.tensor.transpose(
                                trp2[:, 0:d], outT_sb[:, tt * P:(tt + 1) * P],
                                ident[0:d, 0:d])
                            iglob = g2 * nblk + tt
                            nc.vector.tensor_scalar_mul(
                                out_sb[:, tt, :], trp2[:, 0:d],
                                recip_all[:, iglob:iglob + 1])
                        nc.sync.dma_start(
                            out=out_d[h, g2 * BLK:(g2 + 1) * BLK, :].rearrange(
                                "(tt p) d -> p tt d", p=P),
                            in_=out_sb)

                n_p2 = G * TB
                for z in range(TB):
                    p1_block(z)
                    w0 = z * n_p2 // TB
                    w1 = (z + 1) * n_p2 // TB
                    for w in range(w0, w1):
                        p2_step(w)
    nc.compile()
    return nc


def rope_tables(t=T, d=D):
    inv_freq = (1.0 / (10000.0 ** (np.arange(0, d, 2, dtype=np.float32) / np.float32(d)))).astype(np.float32)
    pos = np.arange(t, dtype=np.float32)
    freqs = pos[:, None] * inv_freq[None, :]  # [t, d//2] fp32
    return np.cos(freqs).astype(np.float32), np.sin(freqs).astype(np.float32)


_CACHED = {}


def kernel(q, k, v):
    """Full-input entry point: q,k,v [2,16,2048,64] fp32 -> (out, attn)."""
    from concourse.bass_utils import run_bass_kernel_spmd

    q = np.ascontiguousarray(q, dtype=np.float32)
    k = np.ascontiguousarray(k, dtype=np.float32)
    v = np.ascontiguousarray(v, dtype=np.float32)

    if "nc" not in _CACHED:
        _CACHED["nc"] = build_nc()
        _CACHED["tables"] = rope_tables()
    nc = _CACHED["nc"]
    cos_half, sin_half = _CACHED["tables"]

    qf = q.reshape(B * H, T, D)
    kf = k.reshape(B * H, T, D)
    vf = v.reshape(B * H, T, D)
    in_maps = []
    for c in range(N_CORES):
        sl = slice(c * HEADS_PER_CORE, (c + 1) * HEADS_PER_CORE)
        in_maps.append({
            "q": qf[sl], "k": kf[sl], "v": vf[sl],
            "cos_half": cos_half, "sin_half": sin_half,
        })
    res = run_bass_kernel_spmd(nc, in_maps, list(range(N_CORES)))
    outs = np.concatenate([r["out"] for r in res.results], axis=0)
    attns = np.concatenate([r["attn"] for r in res.results], axis=0)
    return (outs.reshape(B, H, T, D), attns.reshape(B, H, T, T))


if __name__ == "__main__":
    rng = np.random.default_rng(0)
    q = rng.standard_normal((B, H, T, D), dtype=np.float32)
    k = rng.standard_normal((B, H, T, D), dtype=np.float32)
    v = rng.standard_normal((B, H, T, D), dtype=np.float32)
    out, attn = kernel(q, k, v)
    print(out.shape, attn.shape, float(attn[0, 0, 0].sum()))


# revision 17
# speedup vs baseline: 2.0227x; 1.2913x over previous
"""Trainium2 Bass kernel for RoPE + scaled-dot-product attention (returns out AND attn).

Reference semantics (B=2, H=16, T=2048, D=64, fp32):
    qr, kr = rope(q), rope(k)
    scores = qr @ kr^T / sqrt(D)          # [B,H,T,T]
    attn   = softmax(scores, axis=-1)     # output 1
    out    = attn @ v                     # output 2

Sharding: B*H = 32 (b,h) pairs, 4 per NeuronCore across 8 cores; no
cross-core communication.

Per-head on-device plan (T=2048, D=64), v2:
  phase 0: load q,k,v; RoPE on DVE in natural [t,d] layout; TensorE
           transposes with a stride-0 broadcast AP produce Qr^T/Kr^T
           DUPLICATED on both partition halves ([128, T], rows 0-63 ==
           rows 64-127) so S and S^T matmuls run on disjoint PE row
           groups concurrently.
  phase 1 (rows 0-63):  S = Qr^T-blk.T @ Kr^T (float32r), exp on ScalarE
           with fused accum_out row-sums, attn = E * (1/rowsum) split
           across DVE and GpSimd, 1MB attn DMA per t-block.
  phase 2 (rows 64-127): S^T s-blocks, exp#2 -> E^T, out^T accumulated as
           V-blk.T @ E^T with column-group packing (j-parity -> PE col
           groups 0/1), then transpose out^T back and scale by 1/rowsum.
  Phases 1 and 2 are independent given phase 0; emission interleaves them
  so TensorE keeps both row-group streams and ScalarE both exp streams.
"""

import os
import sys

import numpy as np

for _p in ("/opt/trn_rl_repo", "/root/.axon_site/_ro/trn_rl_repo"):
    if os.path.isdir(_p) and _p not in sys.path:
        sys.path.append(_p)

import concourse.bacc as bacc
import concourse.bass as bass
import concourse.mybir as mybir
import concourse.tile as tile
from concourse.masks import make_identity

F32 = mybir.dt.float32
F32R = mybir.dt.float32r
F16 = mybir.dt.float16
EXP = mybir.ActivationFunctionType.Exp

N_CORES = 8
B, H, T, D = 2, 16, 2048, 64
HEADS_PER_CORE = (B * H) // N_CORES  # 4
SCALE = 1.0 / np.sqrt(np.float32(D))  # 1/8


def _bcast2(ap):
    """[P, F] AP read twice along free dim -> [P, 2, F] with stride-0 mid."""
    return bass.AP(tensor=ap.tensor, offset=ap.offset,
                   ap=[ap.ap[0], [0, 2], ap.ap[1]])


def build_nc(heads=HEADS_PER_CORE, t=T, d=D):
    """Emit the per-core Bass program. All shapes static."""
    P = 128
    TB = t // P           # 128-row blocks (16)
    MM_N = min(512, t)    # matmul moving free dim
    BLK = min(1024, t)    # free-dim block for exp / S tiles
    G = t // BLK          # s-halves per t-block
    NCH = BLK // MM_N     # matmul chunks per BLK
    dh = d // 2

    nc = bacc.Bacc("TRN2", target_bir_lowering=False, debug=False)
    q_d = nc.dram_tensor("q", [heads, t, d], F32, kind="ExternalInput")
    k_d = nc.dram_tensor("k", [heads, t, d], F32, kind="ExternalInput")
    v_d = nc.dram_tensor("v", [heads, t, d], F32, kind="ExternalInput")
    cos_d = nc.dram_tensor("cos_half", [t, dh], F32, kind="ExternalInput")
    sin_d = nc.dram_tensor("sin_half", [t, dh], F32, kind="ExternalInput")
    attn_d = nc.dram_tensor("attn", [heads, t, t], F32, kind="ExternalOutput")
    out_d = nc.dram_tensor("out", [heads, t, d], F32, kind="ExternalOutput")

    with tile.TileContext(nc) as tc:
        with (
            tc.tile_pool(name="singles", bufs=1) as singles,
            tc.tile_pool(name="nat", bufs=2) as p_nat,
            tc.tile_pool(name="tmp", bufs=2) as p_tmp,
            tc.tile_pool(name="tmat", bufs=2) as p_tmat,
            tc.tile_pool(name="e", bufs=4) as p_e,
            tc.tile_pool(name="et", bufs=3) as p_et,
            tc.tile_pool(name="attn", bufs=4) as p_attn,
            tc.tile_pool(name="small", bufs=4) as p_small,
            tc.tile_pool(name="head", bufs=2) as p_head,
            tc.tile_pool(name="outb", bufs=2) as p_out,
            tc.tile_pool(name="mm", bufs=1, space="PSUM") as p_mm,
            tc.tile_pool(name="av", bufs=2, space="PSUM") as p_av,
            tc.tile_pool(name="tr", bufs=2, space="PSUM") as p_tr,
        ):
            ident = singles.tile([P, P], F32)
            make_identity(nc, ident)

            cos_sb = singles.tile([P, TB, dh], F32)
            sin_sb = singles.tile([P, TB, dh], F32)
            nc.gpsimd.dma_start(
                out=cos_sb, in_=cos_d.rearrange("(i p) d -> p i d", p=P))
            nc.gpsimd.dma_start(
                out=sin_sb, in_=sin_d.rearrange("(i p) d -> p i d", p=P))

            for h in range(heads):
                # ---------------- phase 0: load + rope + transpose ----------
                q_nat = p_nat.tile([P, TB, d], F32, tag="q_nat")
                k_nat = p_nat.tile([P, TB, d], F32, tag="k_nat")
                v_sb = p_nat.tile([P, TB, d], F16, tag="v_sb")
                nc.gpsimd.dma_start(
                    out=q_nat, in_=q_d[h].rearrange("(i p) d -> p i d", p=P))
                nc.gpsimd.dma_start(
                    out=k_nat, in_=k_d[h].rearrange("(i p) d -> p i d", p=P))
                nc.gpsimd.dma_start(
                    out=v_sb, in_=v_d[h].rearrange("(i p) d -> p i d", p=P))

                qr_nat = p_nat.tile([P, TB, d], F32, tag="qr_nat")
                kr_nat = p_nat.tile([P, TB, d], F32, tag="kr_nat")
                for src, dst, tg in ((q_nat, qr_nat, "tq"), (k_nat, kr_nat, "tk")):
                    x1 = src[:, :, 0:dh]
                    x2 = src[:, :, dh:d]
                    tmp = p_tmp.tile([P, TB, dh], F32, tag=tg)
                    nc.vector.tensor_mul(dst[:, :, 0:dh], x1, cos_sb)
                    nc.vector.tensor_mul(tmp, x2, sin_sb)
                    nc.vector.tensor_sub(dst[:, :, 0:dh], dst[:, :, 0:dh], tmp)
                    nc.vector.tensor_mul(dst[:, :, dh:d], x1, sin_sb)
                    nc.vector.tensor_mul(tmp, x2, cos_sb)
                    nc.vector.tensor_add(dst[:, :, dh:d], dst[:, :, dh:d], tmp)

                # Qr^T / Kr^T duplicated on both partition halves: PE
                # transpose into rows 0-63, then one SB->SB DMA up-copy.
                qrT = p_tmat.tile([P, t], F32R, tag="qrT")
                krT = p_tmat.tile([P, t], F32R, tag="krT")
                for mat, matT in ((qr_nat, qrT), (kr_nat, krT)):
                    for i in range(TB):
                        trp = p_tr.tile([P, P], F32, tag="tr")
                        nc.tensor.transpose(trp[0:d, :], mat[:, i, :], ident)
                        nc.vector.tensor_copy(
                            matT[0:d, i * P:(i + 1) * P], trp[0:d, :])
                    nc.sync.dma_start(out=matT[d:2 * d, :], in_=matT[0:d, :])

                recip_all = p_head.tile([P, TB], F32, tag="recip")

                # Phase-1 t-block i and phase-2 (g2, j) steps, interleaved at
                # MATMUL granularity: S on PE rows 0-63, S^T on rows 64-127,
                # so adjacent queue entries overlap in the array.
                av_state = {}

                def s_mm(i, g, c, s_ps):
                    nc.tensor.matmul(
                        s_ps[:, c * MM_N:(c + 1) * MM_N],
                        qrT[0:d, i * P:(i + 1) * P],
                        krT[0:d, g * BLK + c * MM_N: g * BLK + (c + 1) * MM_N],
                        start=True, stop=True)

                def st_mm(g2, j, c, st_ps):
                    nc.tensor.matmul(
                        st_ps[:, c * MM_N:(c + 1) * MM_N],
                        krT[64:64 + d, j * P:(j + 1) * P],
                        qrT[64:64 + d,
                            g2 * BLK + c * MM_N: g2 * BLK + (c + 1) * MM_N],
                        start=True, stop=True)

                def p1_exp(i, g, s_ps, racc):
                    e_sb = p_e.tile([P, BLK], F32, tag="e")
                    nc.scalar.activation(
                        e_sb, s_ps, EXP, scale=float(SCALE),
                        accum_out=racc[:, g:g + 1])
                    return e_sb

                def p1_finish(i, racc, e_blocks):
                    r_i = p_small.tile([P, 1], F32, tag="r")
                    if G == 1:
                        nc.vector.reciprocal(recip_all[:, i:i + 1], racc[:, 0:1])
                    else:
                        nc.vector.tensor_add(r_i, racc[:, 0:1], racc[:, 1:2])
                        for g in range(2, G):
                            nc.vector.tensor_add(r_i, r_i, racc[:, g:g + 1])
                        nc.vector.reciprocal(recip_all[:, i:i + 1], r_i)
                    attn_sb = p_attn.tile([P, t], F32, tag="attn")
                    for g in range(G):
                        nc.vector.tensor_scalar_mul(
                            attn_sb[:, g * BLK:(g + 1) * BLK], e_blocks[g],
                            recip_all[:, i:i + 1])
                    nc.sync.dma_start(
                        out=attn_d[h, i * P:(i + 1) * P, :], in_=attn_sb)

                def p2_av(g2, j, et_sb):
                    # chunk c -> PE col group c; accumulated over all j.
                    av_ps = av_state["av"]
                    for c in range(NCH):
                        nc.tensor.matmul(
                            av_ps[c * d:(c + 1) * d, :],
                            v_sb[:, j, :],
                            et_sb[:, c * MM_N:(c + 1) * MM_N],
                            start=(j == 0), stop=(j == TB - 1),
                            tile_position=(0, c * 64))

                def p2_finish(g2):
                    av_ps = av_state["av"]
                    outT_sb = p_out.tile([d, BLK], F32, tag="outT")
                    for c in range(NCH):
                        nc.vector.tensor_copy(
                            outT_sb[:, c * MM_N:(c + 1) * MM_N],
                            av_ps[c * d:(c + 1) * d, :])
                    nblk = BLK // P
                    out_sb = p_out.tile([P, nblk, d], F32, tag="out_sb")
                    for tt in range(nblk):
                        trp2 = p_tr.tile([P, P], F32, tag="tr")
                        nc.tensor.transpose(
                            trp2[:, 0:d], outT_sb[:, tt * P:(tt + 1) * P],
                            ident[0:d, 0:d])
                        iglob = g2 * nblk + tt
                        nc.vector.tensor_scalar_mul(
                            out_sb[:, tt, :], trp2[:, 0:d],
                            recip_all[:, iglob:iglob + 1])
                    nc.sync.dma_start(
                        out=out_d[h, g2 * BLK:(g2 + 1) * BLK, :].rearrange(
                            "(tt p) d -> p tt d", p=P),
                        in_=out_sb)

                # z-th macro step: phase-1 block i=z and G phase-2 steps.
                for z in range(TB):
                    racc = p_small.tile([P, max(G, 2)], F32, tag="racc")
                    e_blocks = []
                    pending_finish = []
                    for g in range(G):
                        w = z * G + g
                        g2, j = divmod(w, TB)
                        if j == 0:
                            av_state["av"] = p_av.tile(
                                [P, MM_N], F32, tag="av", name="av_ps")
                        s_ps = p_mm.tile([P, BLK], F32, tag="mm")
                        st_ps = p_mm.tile([P, BLK], F32, tag="mm2")
                        for c in range(NCH):
                            s_mm(z, g, c, s_ps)
                            st_mm(g2, j, c, st_ps)
                        e_blocks.append(p1_exp(z, g, s_ps, racc))
                        et_sb = p_et.tile([P, BLK], F16, tag="et")
                        nc.scalar.activation(
                            et_sb, st_ps, EXP, scale=float(SCALE))
                        p2_av(g2, j, et_sb)
                        if j == TB - 1:
                            pending_finish.append(g2)
                    p1_finish(z, racc, e_blocks)
                    for g2 in pending_finish:
                        p2_finish(g2)
    nc.compile()
    return nc


def rope_tables(t=T, d=D):
    inv_freq = (1.0 / (10000.0 ** (np.arange(0, d, 2, dtype=np.float32) / np.float32(d)))).astype(np.float32)
    pos = np.arange(t, dtype=np.float32)
    freqs = pos[:, None] * inv_freq[None, :]  # [t, d//2] fp32
    return np.cos(freqs).astype(np.float32), np.sin(freqs).astype(np.float32)


_CACHED = {}


def kernel(q, k, v):
    """Full-input entry point: q,k,v [2,16,2048,64] fp32 -> (out, attn)."""
    from concourse.bass_utils import run_bass_kernel_spmd

    q = np.ascontiguousarray(q, dtype=np.float32)
    k = np.ascontiguousarray(k, dtype=np.float32)
    v = np.ascontiguousarray(v, dtype=np.float32)

    if "nc" not in _CACHED:
        _CACHED["nc"] = build_nc()
        _CACHED["tables"] = rope_tables()
    nc = _CACHED["nc"]
    cos_half, sin_half = _CACHED["tables"]

    qf = q.reshape(B * H, T, D)
    kf = k.reshape(B * H, T, D)
    vf = v.reshape(B * H, T, D)
    in_maps = []
    for c in range(N_CORES):
        sl = slice(c * HEADS_PER_CORE, (c + 1) * HEADS_PER_CORE)
        in_maps.append({
            "q": qf[sl], "k": kf[sl], "v": vf[sl],
            "cos_half": cos_half, "sin_half": sin_half,
        })
    res = run_bass_kernel_spmd(nc, in_maps, list(range(N_CORES)))
    outs = np.concatenate([r["out"] for r in res.results], axis=0)
    attns = np.concatenate([r["attn"] for r in res.results], axis=0)
    return (outs.reshape(B, H, T, D), attns.reshape(B, H, T, T))


if __name__ == "__main__":
    rng = np.random.default_rng(0)
    q = rng.standard_normal((B, H, T, D), dtype=np.float32)
    k = rng.standard_normal((B, H, T, D), dtype=np.float32)
    v = rng.standard_normal((B, H, T, D), dtype=np.float32)
    out, attn = kernel(q, k, v)
    print(out.shape, attn.shape, float(attn[0, 0, 0].sum()))
